# revision 1
# baseline (speedup 1.0000x reference)
"""Trainium2 Bass kernel for MoE soft-routed classification head.

Reference math (B=32, S=128, H=1024, E=16, L=8):
    sel_dw = einsum('be,eoh->boh', gates, dense_w)
    sel_db = einsum('be,eh->bh',  gates, dense_b)
    sel_ow = einsum('be,elh->blh', gates, out_proj_w)
    sel_ob = einsum('be,el->bl',  gates, out_proj_b)
    x   = X[:, 0, :]
    h   = tanh(einsum('bh,boh->bo', x, sel_dw) + sel_db)
    out = einsum('bh,blh->bl', h, sel_ow) + sel_ob

Key reordering:
  h_pre[b,o] = sum_{e,h} (gates[b,e]*x[b,h]) * dense_w[e,o,h]
             + sum_e gates[b,e]*dense_b[e,o]
so with Z[(e,h),b] = gates[b,e]*x[b,h] (plus E extra rows equal to gates for
the bias) stage 1 is ONE matmul with contraction K = E*H + E, and only the
CLS token of X is ever touched.

Sharding: dense_w's output dim `o` (H=1024) is split 128-per-core across 8
cores.  Each core computes h[:, o_slice] end-to-end (matmul + bias + tanh),
then its partial contribution to the final [B,L] output:
  part_k[b,l] = sum_e gates[b,e] * sum_{o in slice} h[b,o]*out_proj_w[e,l,o]
The host sums the 8 [32,8] partials and adds gates@out_proj_b.

Data streams to the PE as fp16 (measured end-to-end rel-err ~4e-4 vs the
fp32 reference), halving HBM traffic vs fp32; PSUM accumulation is fp32.
The Z and W chunk streams are interleaved into one DRAM tensor so each DMA
carries matched lhsT/rhs chunk groups; groups alternate between the two
HWDGE queues (SP + Activation) to use both hardware DMA paths.
"""

import contextlib
import ctypes
import os
import sys
import types

import numpy as np


def _install_ntff_shim():
    """Provide antenv.axon_hooks if the image's antenv lacks it.

    bass_utils' trace path does ``from antenv.axon_hooks import
    get_axon_ntff_profile_hook`` and crashes when the module is absent;
    pre-seeding sys.modules with a ctypes equivalent of
    trn_agent_boot.trn_boot._ntff_profile_via_ctypes restores profiling.
    """
    try:
        import antenv.axon_hooks  # noqa: F401
        return
    except ImportError:
        pass

    so_path = "/opt/axon/libaxon_pjrt.so"
    hook = None
    if os.path.exists(so_path):
        try:
            lib = ctypes.CDLL(so_path)
            if hasattr(lib, "axon_start_nrt_profile"):
                lib.axon_start_nrt_profile.argtypes = [
                    ctypes.POINTER(ctypes.c_int64), ctypes.c_size_t]
                lib.axon_start_nrt_profile.restype = ctypes.c_int64
                lib.axon_stop_nrt_profile.argtypes = [ctypes.c_char_p]
                lib.axon_stop_nrt_profile.restype = ctypes.c_int64

                @contextlib.contextmanager
                def _hook(output_dir, device_ids):
                    import jax
                    jax.devices()
                    if device_ids:
                        ids = (ctypes.c_int64 * len(device_ids))(*device_ids)
                        rc = lib.axon_start_nrt_profile(ids, len(device_ids))
                    else:
                        rc = lib.axon_start_nrt_profile(None, 0)
                    if rc != 0:
                        raise RuntimeError(f"axon_start_nrt_profile rc={rc}")
                    try:
                        yield
                    finally:
                        n = lib.axon_stop_nrt_profile(str(output_dir).encode())
                        print(f"ntff profile: {n} file(s) -> {output_dir}",
                              file=sys.stderr)

                hook = _hook
        except OSError:
            pass

    mod = types.ModuleType("antenv.axon_hooks")
    mod._hook = hook
    mod.set_axon_ntff_profile_hook = lambda h: setattr(mod, "_hook", h)
    mod.get_axon_ntff_profile_hook = lambda: mod._hook
    sys.modules["antenv.axon_hooks"] = mod


_install_ntff_shim()

B, S, H, E, L = 32, 128, 1024, 16, 8
NCORES = 8
OSL = H // NCORES            # 128 output columns of dense layer per core
KTOT = E * H + E             # 16400 contraction rows (incl. bias rows)
NCH = (KTOT + 127) // 128    # 129 K-chunks of 128
KPAD = NCH * 128             # 16512
EL = E * L                   # 128
NHC = H // 128               # 8 x-chunks
# xg packed input layout (fp16, [128, XGW]): x chunks | broadcast gates | bias-z
XG_XT = 0                    # xt[p, hc*B+b] = x[b, hc*128+p]      (NHC*B cols)
XG_G = NHC * B               # g128[p, e*B+b] = gates[b, e]        (E*B cols)
XG_ZT = XG_G + E * B         # ztail[p, b] = gates[b, p] if p < E  (B cols)
XGW = XG_ZT + B              # 800
# DMA chunk-groups as (start_chunk, count, engine): the SP queue carries
# chunks 0-63 + the bias chunk, the Activation queue chunks 64-127; groups
# are emitted alternately so the PE consumes in arrival order (the PSUM
# accumulation is commutative) and both queues stay busy end-to-end.
GROUPS = [(0, 8, "s"), (64, 16, "a"), (8, 20, "s"), (80, 16, "a"),
          (28, 20, "s"), (96, 16, "a"), (48, 16, "s"), (112, 16, "a"),
          (128, 1, "s")]
assert sum(n for _, n, _ in GROUPS) == NCH
GMAX = max(n for _, n, _ in GROUPS)

_CACHE = {}

# Results of the most recent hardware run (BassKernelResults); harnesses can
# read .exec_time_ns when run with BASS_TRACE=1.
LAST_RESULTS = None


def _build_nc():
    import concourse.bacc as bacc
    import concourse.tile as tile
    import concourse.mybir as mybir

    f16 = mybir.dt.float16
    f32 = mybir.dt.float32

    nc = bacc.Bacc("TRN2", target_bir_lowering=False, debug=False,
                   num_devices=NCORES)

    w_d = nc.dram_tensor("w", [128, NCH * OSL], f16, kind="ExternalInput")
    xg_d = nc.dram_tensor("xg", [128, XGW], f16, kind="ExternalInput")
    ow_d = nc.dram_tensor("ow", [OSL, EL], f16, kind="ExternalInput")
    gex_d = nc.dram_tensor("gex", [B, EL], f32, kind="ExternalInput")
    out_d = nc.dram_tensor("out", [B, L], f32, kind="ExternalOutput")

    with tile.TileContext(nc) as tc:
        with (
            tc.tile_pool(name="const", bufs=1) as cpool,
            tc.tile_pool(name="wzp", bufs=10) as wzp,
            tc.tile_pool(name="work", bufs=1) as spool,
            tc.tile_pool(name="psum", bufs=1, space="PSUM") as ppool,
        ):
            # xg: x chunks + broadcast gates + bias-z rows.  First transfer on
            # the Activation queue — small, so Z generation starts while the
            # first weight groups stream.
            xg_sb = cpool.tile([128, XGW], f16)
            nc.scalar.dma_start(xg_sb[:], xg_d[:])

            # Z on device: zt[p, c*B+b] = gates[b, c//NHC-block] * x chunks.
            # One DVE multiply per expert over its NHC chunks, the gates
            # operand broadcast across the chunk dim via a step-0 AP.
            # Experts ordered to match the paired A/B chunk consumption.
            zt_sb = spool.tile([128, (NCH - 1) * B], f16)
            xt3 = xg_sb[:, XG_XT : XG_XT + NHC * B].rearrange(
                "p (h b) -> p h b", b=B)
            for e in [0, 8, 1, 9, 2, 10, 3, 11, 4, 12, 5, 13, 6, 14, 7, 15]:
                g_b = (
                    xg_sb[:, XG_G + e * B : XG_G + (e + 1) * B]
                    .unsqueeze(1)
                    .to_broadcast((128, NHC, B))
                )
                nc.vector.tensor_mul(
                    zt_sb[:, e * NHC * B : (e + 1) * NHC * B].rearrange(
                        "p (h b) -> p h b", b=B),
                    xt3,
                    g_b,
                )

            # Stage 1: hT_pre[o, b] accumulated over 129 K-chunks; the W part
            # of each chunk is the stationary operand so the result lands
            # o-major and stage 2 needs no transpose.
            ps1 = ppool.tile([OSL, B], f32)
            engines = {"s": nc.sync, "a": nc.scalar}
            for g, (cs, n_c, ename) in enumerate(GROUPS):
                eng = engines[ename]
                wt = wzp.tile([128, GMAX * OSL], f16, tag="wt")
                eng.dma_start(
                    wt[:, : n_c * OSL],
                    w_d[:, cs * OSL : (cs + n_c) * OSL],
                )
                for i in range(n_c):
                    c = cs + i
                    rhs = (
                        zt_sb[:, c * B : (c + 1) * B]
                        if c < NCH - 1
                        else xg_sb[:, XG_ZT : XG_ZT + B]
                    )
                    nc.tensor.matmul(
                        ps1[:],
                        wt[:, i * OSL : (i + 1) * OSL],
                        rhs,
                        start=(g == 0 and i == 0),
                        stop=(g == len(GROUPS) - 1 and i == n_c - 1),
                    )

            ow_sb = cpool.tile([OSL, EL], f16)
            nc.scalar.dma_start(ow_sb[:], ow_d[:])
            gex_sb = cpool.tile([B, EL], f32)
            nc.scalar.dma_start(gex_sb[:], gex_d[:])

            ht = spool.tile([OSL, B], f16)
            nc.scalar.activation(ht[:], ps1[:],
                                 mybir.ActivationFunctionType.Tanh)

            ps2 = ppool.tile([B, EL], f32)
            nc.tensor.matmul(ps2[:], ht[:], ow_sb[:], start=True, stop=True)

            # r[b, (l,e)] = ps2 * gates[b,e]; reduce over e (innermost).
            r = spool.tile([B, EL], f32)
            nc.vector.tensor_mul(r[:], ps2[:], gex_sb[:])
            out_r = spool.tile([B, L], f32)
            nc.vector.tensor_reduce(
                out_r[:],
                r[:].rearrange("p (l e) -> p l e", e=E),
                axis=mybir.AxisListType.X,
                op=mybir.AluOpType.add,
            )
            nc.sync.dma_start(out_d[:], out_r[:])

    nc.compile()
    return nc


def _get_nc():
    if "nc" not in _CACHE:
        _CACHE["nc"] = _build_nc()
    return _CACHE["nc"]


def make_in_maps(X, gates, dense_w, dense_b, out_proj_w, out_proj_b):
    """Host-side shard + pack. Returns (in_maps, host_bias)."""
    X = np.asarray(X, np.float32)
    gates = np.asarray(gates, np.float32)
    dense_w = np.asarray(dense_w, np.float32)
    dense_b = np.asarray(dense_b, np.float32)
    out_proj_w = np.asarray(out_proj_w, np.float32)
    out_proj_b = np.asarray(out_proj_b, np.float32)

    x = X[:, 0, :]                                     # [B, H]

    # xg packed input: x chunks | gates broadcast over partitions | bias-z
    xg = np.zeros((128, XGW), np.float16)
    # xt[p, hc*B+b] = x[b, hc*128+p]
    xg[:, XG_XT : XG_XT + NHC * B] = (
        x.T.reshape(NHC, 128, B).transpose(1, 0, 2).reshape(128, NHC * B)
    )
    xg[:, XG_G : XG_G + E * B] = np.broadcast_to(
        gates.T.reshape(1, E * B), (128, E * B)
    )
    xg[:E, XG_ZT : XG_ZT + B] = gates.T               # bias-z rows

    dw_t = dense_w.transpose(0, 2, 1)                  # [E, h, o]
    # l-major expert expansion: gex[b, l*E+e] = gates[b, e]
    gex = np.ascontiguousarray(
        np.tile(gates, (1, L)).astype(np.float32)
    )

    in_maps = []
    for k in range(NCORES):
        sl = slice(k * OSL, (k + 1) * OSL)
        w = np.zeros((KPAD, OSL), np.float32)
        w[: E * H] = dw_t[:, :, sl].reshape(E * H, OSL)
        w[E * H : E * H + E] = dense_b[:, sl]
        # partition-major for the DMA: w[p, c*OSL + j] = w_chunks[c, p, j]
        w_pk = np.ascontiguousarray(
            w.reshape(NCH, 128, OSL).transpose(1, 0, 2).reshape(128, NCH * OSL)
        ).astype(np.float16)

        # ow[o, l*E+e] = out_proj_w[e, l, o]  (l-major for the final reduce)
        ow = np.ascontiguousarray(
            out_proj_w[:, :, sl].transpose(2, 1, 0).reshape(OSL, EL)
        ).astype(np.float16)

        in_maps.append({"w": w_pk, "xg": xg, "ow": ow, "gex": gex})

    host_bias = (gates @ out_proj_b).astype(np.float32)   # [B, L]
    return in_maps, host_bias


def kernel(**inputs):
    global LAST_RESULTS
    from concourse.bass_utils import run_bass_kernel_spmd

    nc = _get_nc()
    in_maps, host_bias = make_in_maps(
        inputs["X"], inputs["gates"], inputs["dense_w"], inputs["dense_b"],
        inputs["out_proj_w"], inputs["out_proj_b"],
    )
    res = run_bass_kernel_spmd(nc, in_maps, list(range(NCORES)))
    LAST_RESULTS = res
    parts = [r["out"] for r in res.results]
    out = np.sum(parts, axis=0, dtype=np.float64).astype(np.float32) + host_bias
    return out

